# revision 11
# baseline (speedup 1.0000x reference)
"""Relational GCN (3-layer, 8 edge types) + subgraph readout on 8 Trainium2 cores.

Strategy (data-parallel over the 32 graphs, 4 graphs/core):
  * All index work happens on the host: depth scatter-max, per-graph node
    relabeling by descending in-degree, and a global "round" schedule.
    Round k holds the k-th incoming edge of every node; after the degree
    sort each round covers a prefix of node slots, so the per-edge
    segment-sum becomes dense prefix adds on the vector engine.
  * Per layer on-device: g[n,t] = h[n] @ W_rel[l,t] (PE), g -> DRAM,
    dma_gather g[src*8+et] in round order (256B rows), DVE prefix-adds
    into agg, PE-transpose + fused bias/ReLU (ACT), gate matmul + fused
    bias/Sigmoid (ACT), DVE h update.  Final MLP + masked-mean readout
    via small matmuls.
"""

import numpy as np

import concourse.bass as bass
import concourse.bacc as bacc
import concourse.mybir as mybir
import concourse.tile as tile
from concourse.bass_utils import run_bass_kernel_spmd

B, N, S = 32, 2048, 16
D, T, L = 64, 8, 3
TN = 32            # node-type vocab
MAXDEP = 31
SELF_LOOP = 1
NCORES = 8
GPC = B // NCORES  # graphs per core
NB = N // 128      # 128-node blocks per graph
ZROW = N * T       # index of the zero row in g
CHUNK_MAX = 4096   # max gather indices per dma_gather call

F32 = mybir.dt.float32
I16 = mybir.dt.int16

_compiled = {}  # (sched, chunks) -> nc


# --------------------------------------------------------------------------
# host preprocessing
# --------------------------------------------------------------------------

def _preprocess(batch_nodes, batch_adj_tuples, subgraph_mask):
    bat = np.asarray(batch_adj_tuples, dtype=np.int64)
    bid, src, et, dst, dep = (bat[:, i] for i in range(5))
    flat_src = bid * N + src
    flat_dst = bid * N + dst

    # depth scatter-max over self-loop-type edges (mirrors the jax scatter)
    sl = (et == SELF_LOOP) & (flat_src >= 0) & (flat_src < B * N)
    depths = np.zeros(B * N, np.int64)
    np.maximum.at(depths, flat_src[sl], dep[sl])
    depths = np.minimum(depths, MAXDEP)

    tn = np.clip(np.asarray(batch_nodes, np.int64).reshape(-1), 0, TN - 1)

    # edges kept: valid type and in-range dst (jax segment_sum drops OOB dst);
    # src is clamped (jax gather clamps)
    keep = (et >= 0) & (et < T) & (flat_dst >= 0) & (flat_dst < B * N)
    e_fs = np.clip(flat_src[keep], 0, B * N - 1)
    e_fd = flat_dst[keep]
    e_et = et[keep]

    # self-loop elision: when every node carries exactly one
    # (d, SELF_LOOP, d) edge, that contribution is g[d, SELF_LOOP] for
    # every d -- fold it into the agg initialization on-chip and drop the
    # edges from the gather stream.
    is_id = (e_et == SELF_LOOP) & (e_fs == e_fd)
    idc = np.bincount(e_fd[is_id], minlength=B * N)
    elide = bool((idc == 1).all())
    if elide:
        e_fs, e_fd, e_et = e_fs[~is_id], e_fd[~is_id], e_et[~is_id]
    e_g = e_fd // N  # owning graph (assumes graph-local edges, as generated)

    perms, degs, edge_data = [], [], []
    for g in range(B):
        m = e_g == g
        ld = (e_fd[m] - g * N).astype(np.int64)
        ls = np.clip(e_fs[m] - g * N, 0, N - 1)
        tt = e_et[m]
        deg = np.bincount(ld, minlength=N)
        perm = np.argsort(-deg, kind="stable")
        rank = np.empty(N, np.int64)
        rank[perm] = np.arange(N)
        perms.append(perm)
        degs.append(deg[perm])  # descending
        edge_data.append((rank[ld], rank[ls], tt))

    # global round schedule (shared by every graph -> one NEFF)
    R = int(max(d[0] for d in degs)) if len(degs) else 1
    ks = np.arange(max(R, 1))
    nk = np.zeros(len(ks), np.int64)
    for dg in degs:
        nk = np.maximum(nk, (dg[None, :] > ks[:, None]).sum(1))
    sched = np.maximum((nk + 127) // 128 * 128, 0)
    if not elide:
        sched[0] = N  # full round 0 initializes agg via copy
    sched = [int(x) for x in sched if x > 0]
    cum = np.concatenate([[0], np.cumsum(sched)]).astype(np.int64)
    tot = int(cum[-1])

    # group whole rounds into gather chunks
    chunks = []  # (stream_off, length, [(round_k, off_in_chunk), ...])
    cur_off, cur_len, cur_rounds = 0, 0, []
    for k, skd in enumerate(sched):
        if cur_len + skd > CHUNK_MAX and cur_len > 0:
            chunks.append((cur_off, cur_len, cur_rounds))
            cur_off, cur_len, cur_rounds = int(cum[k]), 0, []
        cur_rounds.append((k, cur_len))
        cur_len += skd
    if cur_len:
        chunks.append((cur_off, cur_len, cur_rounds))

    # per-graph gather streams
    streams = np.full((B, tot), ZROW, np.int16)
    for g in range(B):
        rd, rs, tt = edge_data[g]
        order = np.argsort(rd, kind="stable")
        rd_s = rd[order]
        val_s = (rs[order] * T + tt[order]).astype(np.int16)
        startd = np.searchsorted(rd_s, np.arange(N))
        k_of = np.arange(len(rd_s)) - startd[rd_s]
        streams[g, cum[k_of] + rd_s] = val_s

    # one-hot embedding selectors, [64, GPC*N] per core
    onehots = np.zeros((NCORES, 2 * TN, GPC * N), np.float32)
    maskT = np.zeros((NCORES, GPC, N, S), np.float32)
    sm = np.asarray(subgraph_mask, np.float32)
    for c in range(NCORES):
        for b in range(GPC):
            g = c * GPC + b
            p = perms[g]
            cols = b * N + np.arange(N)
            onehots[c, tn[g * N + p], cols] = 1.0
            onehots[c, TN + depths[g * N + p], cols] = 1.0
            maskT[c, b] = sm[g].T[p, :]

    # packed int16 index image per core: [128, GPC*tot/16]
    idx_imgs = np.zeros((NCORES, 128, GPC * tot // 16), np.int16)
    for c in range(NCORES):
        st = streams[c * GPC:(c + 1) * GPC].reshape(-1)
        idx_imgs[c] = np.tile(st.reshape(-1, 16).T, (8, 1))

    return (tuple(sched), tuple(
        (o, l, tuple(r)) for o, l, r in chunks
    ), elide), tot, idx_imgs, onehots, maskT


# --------------------------------------------------------------------------
# device kernel
# --------------------------------------------------------------------------

def _build(sched, chunks, tot, elide):
    import os
    n_graphs = int(os.environ.get("KB_GRAPHS", GPC))
    n_layers = int(os.environ.get("KB_LAYERS", L))
    stage = int(os.environ.get("KB_STAGE", 9))
    nc = bacc.Bacc("TRN2", target_bir_lowering=False, debug=False,
                   num_devices=NCORES, num_swdge_queues=4)
    qrr = [0]  # round-robin gather queue across the 4 SWDGE queue pairs

    idx_in = nc.dram_tensor("idximg", [128, GPC * tot // 16], I16,
                            kind="ExternalInput")
    oh_in = nc.dram_tensor("onehot", [2 * TN, GPC * N], F32,
                           kind="ExternalInput")
    bd_in = nc.dram_tensor("blockdiag", [D, D], F32, kind="ExternalInput")
    wrel_in = nc.dram_tensor("wrel", [L * T * D, D], F32, kind="ExternalInput")
    wgate_in = nc.dram_tensor("wgate", [L * D, D], F32, kind="ExternalInput")
    wout_in = nc.dram_tensor("wout", [2 * D, D], F32, kind="ExternalInput")
    bconv_in = nc.dram_tensor("bconvT", [D, L], F32, kind="ExternalInput")
    bgate_in = nc.dram_tensor("bgateT", [D, L], F32, kind="ExternalInput")
    bout_in = nc.dram_tensor("boutT", [D, 2], F32, kind="ExternalInput")
    b1rep_in = nc.dram_tensor("b1rep", [128, D], F32, kind="ExternalInput")
    ident_in = nc.dram_tensor("ident", [128, 128], F32, kind="ExternalInput")
    mask_in = nc.dram_tensor("maskT", [GPC, N, S], F32, kind="ExternalInput")
    gemb_out = nc.dram_tensor("gemb", [GPC, S, D], F32, kind="ExternalOutput")

    with tile.TileContext(nc) as tc:
        with (
            tc.tile_pool(name="const", bufs=1) as cp,
            tc.tile_pool(name="work", bufs=3) as wp,
            tc.tile_pool(name="chunk", bufs=4) as ckp,
            tc.tile_pool(name="aggp", bufs=4) as agp,
            tc.tile_pool(name="hTp", bufs=6) as hp,
            tc.tile_pool(name="acp", bufs=4) as acp,
            tc.tile_pool(name="psum", bufs=2, space=bass.MemorySpace.PSUM) as pp,
            tc.tile_pool(name="psacc", bufs=2, space=bass.MemorySpace.PSUM) as pa,
            tc.tile_pool(name="gdram", bufs=4, space=bass.MemorySpace.DRAM) as gp,
        ):
            # ---- resident constants ----
            idx_t = cp.tile([128, GPC * tot // 16], I16)
            nc.sync.dma_start(out=idx_t[:], in_=idx_in[:])
            bd_t = cp.tile([D, D], F32)
            nc.sync.dma_start(out=bd_t[:], in_=bd_in[:])
            wrel_t = cp.tile([D, L * T, D], F32)
            nc.sync.dma_start(
                out=wrel_t[:],
                in_=wrel_in[:].rearrange("(lt k) n -> k lt n", k=D))
            wgate_t = cp.tile([D, L, D], F32)
            nc.sync.dma_start(
                out=wgate_t[:],
                in_=wgate_in[:].rearrange("(l k) n -> k l n", k=D))
            wout_t = cp.tile([D, 2, D], F32)
            nc.sync.dma_start(
                out=wout_t[:],
                in_=wout_in[:].rearrange("(l k) n -> k l n", k=D))
            bconv_t = cp.tile([D, L], F32)
            nc.sync.dma_start(out=bconv_t[:], in_=bconv_in[:])
            bgate_t = cp.tile([D, L], F32)
            nc.sync.dma_start(out=bgate_t[:], in_=bgate_in[:])
            bout_t = cp.tile([D, 2], F32)
            nc.sync.dma_start(out=bout_t[:], in_=bout_in[:])
            b1rep_t = cp.tile([128, D], F32)
            nc.sync.dma_start(out=b1rep_t[:], in_=b1rep_in[:])
            ident_t = cp.tile([128, 128], F32)
            nc.sync.dma_start(out=ident_t[:], in_=ident_in[:])
            mask_t = cp.tile([128, GPC * NB, S], F32)
            for b in range(GPC):
                nc.sync.dma_start(
                    out=mask_t[:, b * NB:(b + 1) * NB, :],
                    in_=mask_in[b].rearrange("(c p) s -> p c s", p=128))
            ones_t = cp.tile([128, 1], F32)
            nc.gpsimd.memset(ones_t[:], 1.0)
            zero_t = cp.tile([128, T * D], F32)
            nc.gpsimd.memset(zero_t[:], 0.0)

            for b in range(n_graphs):
                # ---- embedding: hT[d, n] for this graph ----
                hT = hp.tile([D, N], F32, tag="hT")
                for c0 in range(0, N, 512):
                    ohc = wp.tile([2 * TN, 512], F32, tag="ohc")
                    nc.sync.dma_start(
                        out=ohc[:], in_=oh_in[:, b * N + c0:b * N + c0 + 512])
                    he = pp.tile([D, 512], F32, tag="psA")
                    nc.tensor.matmul(he[:], bd_t[:], ohc[:],
                                     start=True, stop=True)
                    nc.vector.tensor_copy(hT[:, c0:c0 + 512], he[:])

                for l in range(n_layers):
                    # ---- g[n, t*64:] = h @ W_rel[l, t] ----
                    g_t = gp.tile([NB + 1, 128 * T * D], F32, tag="g")
                    nc.sync.dma_start(
                        out=g_t[NB].rearrange("(p f) -> p f", p=128),
                        in_=zero_t[:])
                    agg = agp.tile([128, NB, D], F32, tag="agg")
                    for c in range(NB):
                        psg = pp.tile([128, T * D], F32, tag="psA")
                        nc.tensor.matmul(
                            psg[:], hT[:, c * 128:(c + 1) * 128],
                            wrel_t[:, l * T:(l + 1) * T, :],
                            start=True, stop=True)
                        gsb = wp.tile([128, T * D], F32, tag="gsb")
                        nc.vector.tensor_copy(gsb[:], psg[:])
                        if elide:
                            nc.vector.tensor_copy(
                                agg[:, c, :],
                                gsb[:, SELF_LOOP * D:(SELF_LOOP + 1) * D])
                        nc.sync.dma_start(
                            out=g_t[c].rearrange("(p f) -> p f", p=128),
                            in_=gsb[:])

                    if stage < 3:
                        continue
                    # ---- gather rounds + prefix adds into agg ----
                    g_flat = g_t[:].rearrange("a (b d) -> (a b) d", d=D)
                    ibase = b * tot
                    for (coff, clen, rounds) in chunks:
                        gt = ckp.tile([128, clen // 128, D], F32, tag="gchunk")
                        nc.gpsimd.dma_gather(
                            out_ap=gt[:],
                            in_ap=g_flat,
                            idxs_ap=idx_t[:, (ibase + coff) // 16:
                                          (ibase + coff + clen) // 16],
                            num_idxs=clen,
                            num_idxs_reg=clen,
                            elem_size=D,
                            single_packet=False,
                            queue_num=qrr[0] % 4)
                        qrr[0] += 1
                        for (k, off) in rounds:
                            bk = sched[k] // 128
                            if k == 0 and not elide:
                                nc.vector.tensor_copy(
                                    agg[:], gt[:, off // 128:off // 128 + bk, :])
                            else:
                                nc.vector.tensor_add(
                                    agg[:, :bk, :], agg[:, :bk, :],
                                    gt[:, off // 128:off // 128 + bk, :])

                    if stage < 4:
                        continue
                    # ---- act = relu(agg + b_conv), transposed ----
                    actT = acp.tile([D, N], F32, tag="actT")
                    for c in range(NB):
                        tp = pp.tile([D, 128], F32, tag="psB")
                        nc.tensor.transpose(tp[:], agg[:, c, :], ident_t[:])
                        nc.scalar.activation(
                            actT[:, c * 128:(c + 1) * 128], tp[:],
                            mybir.ActivationFunctionType.Relu,
                            bias=bconv_t[:, l:l + 1])

                    # ---- gate = sigmoid(act @ W_gate + b_gate); h update ----
                    hT_new = hp.tile([D, N], F32, tag="hT")
                    for c0 in range(0, N, 512):
                        gps = pp.tile([D, 512], F32, tag="psC")
                        nc.tensor.matmul(
                            gps[:], wgate_t[:, l, :],
                            actT[:, c0:c0 + 512], start=True, stop=True)
                        gateT = wp.tile([D, 512], F32, tag="gateT")
                        nc.scalar.activation(
                            gateT[:], gps[:],
                            mybir.ActivationFunctionType.Sigmoid,
                            bias=bgate_t[:, l:l + 1])
                        nc.vector.tensor_mul(
                            gateT[:], gateT[:], actT[:, c0:c0 + 512])
                        nc.vector.tensor_add(
                            hT_new[:, c0:c0 + 512], gateT[:],
                            hT[:, c0:c0 + 512])
                    hT = hT_new

                if stage < 5:
                    continue
                # ---- output MLP ----
                out1T = acp.tile([D, N], F32, tag="actT")
                for c0 in range(0, N, 512):
                    ops_ = pp.tile([D, 512], F32, tag="psC")
                    nc.tensor.matmul(
                        ops_[:], wout_t[:, 0, :], hT[:, c0:c0 + 512],
                        start=True, stop=True)
                    nc.scalar.activation(
                        out1T[:, c0:c0 + 512], ops_[:],
                        mybir.ActivationFunctionType.Relu,
                        bias=bout_t[:, 0:1])

                # ---- readout: emb[s,d] = sum_n mask[n,s] out2[n,d] ----
                emb_ps = pa.tile([S, D], F32, tag="acc")
                cnt_ps = pa.tile([S, 1], F32, tag="acc")
                for c in range(NB):
                    o2 = pp.tile([128, D], F32, tag="psB")
                    nc.tensor.matmul(
                        o2[:], out1T[:, c * 128:(c + 1) * 128],
                        wout_t[:, 1, :], start=True, stop=True)
                    o2sb = wp.tile([128, D], F32, tag="o2sb")
                    nc.vector.tensor_add(o2sb[:], o2[:], b1rep_t[:])
                    mk = mask_t[:, b * NB + c, :]
                    nc.tensor.matmul(emb_ps[:], mk, o2sb[:],
                                     start=(c == 0), stop=(c == NB - 1))
                    nc.tensor.matmul(cnt_ps[:], mk, ones_t[:],
                                     start=(c == 0), stop=(c == NB - 1))
                cnt_sb = wp.tile([S, 1], F32, tag="cnt")
                nc.vector.tensor_scalar_max(cnt_sb[:], cnt_ps[:], 1.0)
                rec_sb = wp.tile([S, 1], F32, tag="cnt")
                nc.vector.reciprocal(rec_sb[:], cnt_sb[:])
                emb_sb = wp.tile([S, D], F32, tag="emb")
                nc.vector.tensor_scalar_mul(emb_sb[:], emb_ps[:], rec_sb[:])
                nc.sync.dma_start(out=gemb_out[b], in_=emb_sb[:])

    nc.compile()
    return nc


# --------------------------------------------------------------------------
# entry point
# --------------------------------------------------------------------------

def kernel(batch_nodes, batch_adj_tuples, subgraph_mask, num_subgraphs,
           type_table, depth_table, W_rel, b_conv, W_gate, b_gate,
           W_out, b_out):
    key, tot, idx_imgs, onehots, maskT = _preprocess(
        batch_nodes, batch_adj_tuples, subgraph_mask)
    sched, chunks, elide = key
    if key not in _compiled:
        _compiled[key] = _build(sched, chunks, tot, elide)
    nc = _compiled[key]

    bd = np.zeros((D, D), np.float32)
    bd[:TN, :TN] = np.asarray(type_table, np.float32)
    bd[TN:, TN:] = np.asarray(depth_table, np.float32)

    common = dict(
        blockdiag=bd,
        wrel=np.asarray(W_rel, np.float32).reshape(L * T * D, D),
        wgate=np.asarray(W_gate, np.float32).reshape(L * D, D),
        wout=np.asarray(W_out, np.float32).reshape(2 * D, D),
        bconvT=np.ascontiguousarray(np.asarray(b_conv, np.float32).T),
        bgateT=np.ascontiguousarray(np.asarray(b_gate, np.float32).T),
        boutT=np.ascontiguousarray(np.asarray(b_out, np.float32).T),
        b1rep=np.tile(np.asarray(b_out, np.float32)[1][None, :], (128, 1)),
        ident=np.eye(128, dtype=np.float32),
    )
    in_maps = [
        dict(common, idximg=idx_imgs[c], onehot=onehots[c], maskT=maskT[c])
        for c in range(NCORES)
    ]

    res = run_bass_kernel_spmd(nc, in_maps, list(range(NCORES)))
    out = np.concatenate([res.results[c]["gemb"] for c in range(NCORES)], 0)
    return out.astype(np.float32)


# revision 12
# speedup vs baseline: 1.0432x; 1.0432x over previous
"""Relational GCN (3-layer, 8 edge types) + subgraph readout on 8 Trainium2 cores.

Strategy (data-parallel over the 32 graphs, 4 graphs/core):
  * All index work happens on the host: depth scatter-max, per-graph node
    relabeling by descending in-degree, and a global "round" schedule.
    Round k holds the k-th incoming edge of every node; after the degree
    sort each round covers a prefix of node slots, so the per-edge
    segment-sum becomes dense prefix adds on the vector engine.
  * Per layer on-device: g[n,t] = h[n] @ W_rel[l,t] (PE), g -> DRAM,
    dma_gather g[src*8+et] in round order (256B rows), DVE prefix-adds
    into agg, PE-transpose + fused bias/ReLU (ACT), gate matmul + fused
    bias/Sigmoid (ACT), DVE h update.  Final MLP + masked-mean readout
    via small matmuls.
"""

import numpy as np

import concourse.bass as bass
import concourse.bacc as bacc
import concourse.mybir as mybir
import concourse.tile as tile
from concourse.bass_utils import run_bass_kernel_spmd

B, N, S = 32, 2048, 16
D, T, L = 64, 8, 3
TN = 32            # node-type vocab
MAXDEP = 31
SELF_LOOP = 1
NCORES = 8
GPC = B // NCORES  # graphs per core
NB = N // 128      # 128-node blocks per graph
ZROW = N * T       # index of the zero row in g
CHUNK_MAX = 4096   # max gather indices per dma_gather call

F32 = mybir.dt.float32
I16 = mybir.dt.int16

_compiled = {}  # (sched, chunks) -> nc


# --------------------------------------------------------------------------
# host preprocessing
# --------------------------------------------------------------------------

def _preprocess(batch_nodes, batch_adj_tuples, subgraph_mask):
    bat = np.asarray(batch_adj_tuples, dtype=np.int64)
    bid, src, et, dst, dep = (bat[:, i] for i in range(5))
    flat_src = bid * N + src
    flat_dst = bid * N + dst

    # depth scatter-max over self-loop-type edges (mirrors the jax scatter)
    sl = (et == SELF_LOOP) & (flat_src >= 0) & (flat_src < B * N)
    depths = np.zeros(B * N, np.int64)
    np.maximum.at(depths, flat_src[sl], dep[sl])
    depths = np.minimum(depths, MAXDEP)

    tn = np.clip(np.asarray(batch_nodes, np.int64).reshape(-1), 0, TN - 1)

    # edges kept: valid type and in-range dst (jax segment_sum drops OOB dst);
    # src is clamped (jax gather clamps)
    keep = (et >= 0) & (et < T) & (flat_dst >= 0) & (flat_dst < B * N)
    e_fs = np.clip(flat_src[keep], 0, B * N - 1)
    e_fd = flat_dst[keep]
    e_et = et[keep]

    # self-loop elision: when every node carries exactly one
    # (d, SELF_LOOP, d) edge, that contribution is g[d, SELF_LOOP] for
    # every d -- fold it into the agg initialization on-chip and drop the
    # edges from the gather stream.
    is_id = (e_et == SELF_LOOP) & (e_fs == e_fd)
    idc = np.bincount(e_fd[is_id], minlength=B * N)
    elide = bool((idc == 1).all())
    if elide:
        e_fs, e_fd, e_et = e_fs[~is_id], e_fd[~is_id], e_et[~is_id]
    e_g = e_fd // N  # owning graph (assumes graph-local edges, as generated)

    perms, degs, edge_data = [], [], []
    for g in range(B):
        m = e_g == g
        ld = (e_fd[m] - g * N).astype(np.int64)
        ls = np.clip(e_fs[m] - g * N, 0, N - 1)
        tt = e_et[m]
        deg = np.bincount(ld, minlength=N)
        perm = np.argsort(-deg, kind="stable")
        rank = np.empty(N, np.int64)
        rank[perm] = np.arange(N)
        perms.append(perm)
        degs.append(deg[perm])  # descending
        edge_data.append((rank[ld], rank[ls], tt))

    # global round schedule (shared by every graph -> one NEFF)
    R = int(max(d[0] for d in degs)) if len(degs) else 1
    ks = np.arange(max(R, 1))
    nk = np.zeros(len(ks), np.int64)
    for dg in degs:
        nk = np.maximum(nk, (dg[None, :] > ks[:, None]).sum(1))
    sched = np.maximum((nk + 127) // 128 * 128, 0)
    if not elide:
        sched[0] = N  # full round 0 initializes agg via copy
    sched = [int(x) for x in sched if x > 0]
    cum = np.concatenate([[0], np.cumsum(sched)]).astype(np.int64)
    tot = int(cum[-1])

    # group whole rounds into gather chunks
    chunks = []  # (stream_off, length, [(round_k, off_in_chunk), ...])
    cur_off, cur_len, cur_rounds = 0, 0, []
    for k, skd in enumerate(sched):
        if cur_len + skd > CHUNK_MAX and cur_len > 0:
            chunks.append((cur_off, cur_len, cur_rounds))
            cur_off, cur_len, cur_rounds = int(cum[k]), 0, []
        cur_rounds.append((k, cur_len))
        cur_len += skd
    if cur_len:
        chunks.append((cur_off, cur_len, cur_rounds))

    # per-graph gather streams
    streams = np.full((B, tot), ZROW, np.int16)
    for g in range(B):
        rd, rs, tt = edge_data[g]
        order = np.argsort(rd, kind="stable")
        rd_s = rd[order]
        val_s = (rs[order] * T + tt[order]).astype(np.int16)
        startd = np.searchsorted(rd_s, np.arange(N))
        k_of = np.arange(len(rd_s)) - startd[rd_s]
        streams[g, cum[k_of] + rd_s] = val_s

    # one-hot embedding selectors, [64, GPC*N] per core
    onehots = np.zeros((NCORES, 2 * TN, GPC * N), np.float32)
    maskT = np.zeros((NCORES, GPC, N, S), np.float32)
    sm = np.asarray(subgraph_mask, np.float32)
    for c in range(NCORES):
        for b in range(GPC):
            g = c * GPC + b
            p = perms[g]
            cols = b * N + np.arange(N)
            onehots[c, tn[g * N + p], cols] = 1.0
            onehots[c, TN + depths[g * N + p], cols] = 1.0
            maskT[c, b] = sm[g].T[p, :]

    # packed int16 index image per core: [128, GPC*tot/16]
    idx_imgs = np.zeros((NCORES, 128, GPC * tot // 16), np.int16)
    for c in range(NCORES):
        st = streams[c * GPC:(c + 1) * GPC].reshape(-1)
        idx_imgs[c] = np.tile(st.reshape(-1, 16).T, (8, 1))

    return (tuple(sched), tuple(
        (o, l, tuple(r)) for o, l, r in chunks
    ), elide), tot, idx_imgs, onehots, maskT


# --------------------------------------------------------------------------
# device kernel
# --------------------------------------------------------------------------

def _build(sched, chunks, tot, elide):
    nc = bacc.Bacc("TRN2", target_bir_lowering=False, debug=False,
                   num_devices=NCORES, num_swdge_queues=4)
    qrr = [0]  # round-robin gather queue across the 4 SWDGE queue pairs

    idx_in = nc.dram_tensor("idximg", [128, GPC * tot // 16], I16,
                            kind="ExternalInput")
    oh_in = nc.dram_tensor("onehot", [2 * TN, GPC * N], F32,
                           kind="ExternalInput")
    bd_in = nc.dram_tensor("blockdiag", [D, D], F32, kind="ExternalInput")
    wrel_in = nc.dram_tensor("wrel", [L * T * D, D], F32, kind="ExternalInput")
    wgate_in = nc.dram_tensor("wgate", [L * D, D], F32, kind="ExternalInput")
    wout_in = nc.dram_tensor("wout", [2 * D, D], F32, kind="ExternalInput")
    bconv_in = nc.dram_tensor("bconvT", [D, L], F32, kind="ExternalInput")
    bgate_in = nc.dram_tensor("bgateT", [D, L], F32, kind="ExternalInput")
    bout_in = nc.dram_tensor("boutT", [D, 2], F32, kind="ExternalInput")
    b1rep_in = nc.dram_tensor("b1rep", [128, D], F32, kind="ExternalInput")
    ident_in = nc.dram_tensor("ident", [128, 128], F32, kind="ExternalInput")
    mask_in = nc.dram_tensor("maskT", [GPC, N, S], F32, kind="ExternalInput")
    gemb_out = nc.dram_tensor("gemb", [GPC, S, D], F32, kind="ExternalOutput")

    with tile.TileContext(nc) as tc:
        with (
            tc.tile_pool(name="const", bufs=1) as cp,
            tc.tile_pool(name="work", bufs=3) as wp,
            tc.tile_pool(name="chunk", bufs=4) as ckp,
            tc.tile_pool(name="aggp", bufs=4) as agp,
            tc.tile_pool(name="hTp", bufs=6) as hp,
            tc.tile_pool(name="acp", bufs=4) as acp,
            tc.tile_pool(name="psum", bufs=2, space=bass.MemorySpace.PSUM) as pp,
            tc.tile_pool(name="psacc", bufs=2, space=bass.MemorySpace.PSUM) as pa,
            tc.tile_pool(name="gdram", bufs=4, space=bass.MemorySpace.DRAM) as gp,
        ):
            # ---- resident constants ----
            idx_t = cp.tile([128, GPC * tot // 16], I16)
            nc.sync.dma_start(out=idx_t[:], in_=idx_in[:])
            bd_t = cp.tile([D, D], F32)
            nc.sync.dma_start(out=bd_t[:], in_=bd_in[:])
            wrel_t = cp.tile([D, L * T, D], F32)
            nc.sync.dma_start(
                out=wrel_t[:],
                in_=wrel_in[:].rearrange("(lt k) n -> k lt n", k=D))
            wgate_t = cp.tile([D, L, D], F32)
            nc.sync.dma_start(
                out=wgate_t[:],
                in_=wgate_in[:].rearrange("(l k) n -> k l n", k=D))
            wout_t = cp.tile([D, 2, D], F32)
            nc.sync.dma_start(
                out=wout_t[:],
                in_=wout_in[:].rearrange("(l k) n -> k l n", k=D))
            bconv_t = cp.tile([D, L], F32)
            nc.sync.dma_start(out=bconv_t[:], in_=bconv_in[:])
            bgate_t = cp.tile([D, L], F32)
            nc.sync.dma_start(out=bgate_t[:], in_=bgate_in[:])
            bout_t = cp.tile([D, 2], F32)
            nc.sync.dma_start(out=bout_t[:], in_=bout_in[:])
            b1rep_t = cp.tile([128, D], F32)
            nc.sync.dma_start(out=b1rep_t[:], in_=b1rep_in[:])
            ident_t = cp.tile([128, 128], F32)
            nc.sync.dma_start(out=ident_t[:], in_=ident_in[:])
            mask_t = cp.tile([128, GPC * NB, S], F32)
            for b in range(GPC):
                nc.sync.dma_start(
                    out=mask_t[:, b * NB:(b + 1) * NB, :],
                    in_=mask_in[b].rearrange("(c p) s -> p c s", p=128))
            ones_t = cp.tile([128, 1], F32)
            nc.gpsimd.memset(ones_t[:], 1.0)
            zero_t = cp.tile([128, T * D], F32)
            nc.gpsimd.memset(zero_t[:], 0.0)

            st = [dict() for _ in range(GPC)]

            def stage_embed(b):
                hT = hp.tile([D, N], F32, tag="hT")
                for c0 in range(0, N, 512):
                    ohc = wp.tile([2 * TN, 512], F32, tag="ohc")
                    nc.sync.dma_start(
                        out=ohc[:], in_=oh_in[:, b * N + c0:b * N + c0 + 512])
                    he = pp.tile([D, 512], F32, tag="psA")
                    nc.tensor.matmul(he[:], bd_t[:], ohc[:],
                                     start=True, stop=True)
                    nc.vector.tensor_copy(hT[:, c0:c0 + 512], he[:])
                st[b]["hT"] = hT

            def g_phase(b, l):
                hT = st[b]["hT"]
                g_t = gp.tile([NB + 1, 128 * T * D], F32, tag="g")
                nc.sync.dma_start(
                    out=g_t[NB].rearrange("(p f) -> p f", p=128), in_=zero_t[:])
                agg = agp.tile([128, NB, D], F32, tag="agg")
                for c in range(NB):
                    psg = pp.tile([128, T * D], F32, tag="psA")
                    nc.tensor.matmul(
                        psg[:], hT[:, c * 128:(c + 1) * 128],
                        wrel_t[:, l * T:(l + 1) * T, :], start=True, stop=True)
                    gsb = wp.tile([128, T * D], F32, tag="gsb")
                    nc.vector.tensor_copy(gsb[:], psg[:])
                    if elide:
                        nc.vector.tensor_copy(
                            agg[:, c, :],
                            gsb[:, SELF_LOOP * D:(SELF_LOOP + 1) * D])
                    nc.sync.dma_start(
                        out=g_t[c].rearrange("(p f) -> p f", p=128), in_=gsb[:])
                st[b]["g_t"], st[b]["agg"] = g_t, agg

            def stage_gather(b, l):
                g_t, agg = st[b]["g_t"], st[b]["agg"]
                g_flat = g_t[:].rearrange("a (b d) -> (a b) d", d=D)
                ibase = b * tot
                for (coff, clen, rounds) in chunks:
                    gt = ckp.tile([128, clen // 128, D], F32, tag="gchunk")
                    nc.gpsimd.dma_gather(
                        out_ap=gt[:], in_ap=g_flat,
                        idxs_ap=idx_t[:, (ibase + coff) // 16:
                                      (ibase + coff + clen) // 16],
                        num_idxs=clen, num_idxs_reg=clen, elem_size=D,
                        single_packet=False, queue_num=qrr[0] % 4)
                    qrr[0] += 1
                    for (k, off) in rounds:
                        bk = sched[k] // 128
                        if k == 0 and not elide:
                            nc.vector.tensor_copy(
                                agg[:], gt[:, off // 128:off // 128 + bk, :])
                        else:
                            nc.vector.tensor_add(
                                agg[:, :bk, :], agg[:, :bk, :],
                                gt[:, off // 128:off // 128 + bk, :])

            def stage_update(b, l):
                hT, agg = st[b]["hT"], st[b]["agg"]
                actT = acp.tile([D, N], F32, tag="actT")
                for c in range(NB):
                    tp = pp.tile([D, 128], F32, tag="psB")
                    nc.tensor.transpose(tp[:], agg[:, c, :], ident_t[:])
                    nc.scalar.activation(
                        actT[:, c * 128:(c + 1) * 128], tp[:],
                        mybir.ActivationFunctionType.Relu,
                        bias=bconv_t[:, l:l + 1])
                hT_new = hp.tile([D, N], F32, tag="hT")
                for c0 in range(0, N, 512):
                    gps = pp.tile([D, 512], F32, tag="psC")
                    nc.tensor.matmul(
                        gps[:], wgate_t[:, l, :], actT[:, c0:c0 + 512],
                        start=True, stop=True)
                    gateT = wp.tile([D, 512], F32, tag="gateT")
                    nc.scalar.activation(
                        gateT[:], gps[:], mybir.ActivationFunctionType.Sigmoid,
                        bias=bgate_t[:, l:l + 1])
                    nc.vector.tensor_mul(
                        gateT[:], gateT[:], actT[:, c0:c0 + 512])
                    nc.vector.tensor_add(
                        hT_new[:, c0:c0 + 512], gateT[:], hT[:, c0:c0 + 512])
                st[b]["hT"] = hT_new
                if l + 1 < L:
                    g_phase(b, l + 1)

            def stage_out(b):
                hT = st[b]["hT"]
                out1T = acp.tile([D, N], F32, tag="actT")
                for c0 in range(0, N, 512):
                    ops_ = pp.tile([D, 512], F32, tag="psC")
                    nc.tensor.matmul(
                        ops_[:], wout_t[:, 0, :], hT[:, c0:c0 + 512],
                        start=True, stop=True)
                    nc.scalar.activation(
                        out1T[:, c0:c0 + 512], ops_[:],
                        mybir.ActivationFunctionType.Relu, bias=bout_t[:, 0:1])
                emb_ps = pa.tile([S, D], F32, tag="acc")
                cnt_ps = pa.tile([S, 1], F32, tag="acc")
                for c in range(NB):
                    o2 = pp.tile([128, D], F32, tag="psB")
                    nc.tensor.matmul(
                        o2[:], out1T[:, c * 128:(c + 1) * 128],
                        wout_t[:, 1, :], start=True, stop=True)
                    o2sb = wp.tile([128, D], F32, tag="o2sb")
                    nc.vector.tensor_add(o2sb[:], o2[:], b1rep_t[:])
                    mk = mask_t[:, b * NB + c, :]
                    nc.tensor.matmul(emb_ps[:], mk, o2sb[:],
                                     start=(c == 0), stop=(c == NB - 1))
                    nc.tensor.matmul(cnt_ps[:], mk, ones_t[:],
                                     start=(c == 0), stop=(c == NB - 1))
                cnt_sb = wp.tile([S, 1], F32, tag="cnt")
                nc.vector.tensor_scalar_max(cnt_sb[:], cnt_ps[:], 1.0)
                rec_sb = wp.tile([S, 1], F32, tag="cnt")
                nc.vector.reciprocal(rec_sb[:], cnt_sb[:])
                emb_sb = wp.tile([S, D], F32, tag="emb")
                nc.vector.tensor_scalar_mul(emb_sb[:], emb_ps[:], rec_sb[:])
                nc.sync.dma_start(out=gemb_out[b], in_=emb_sb[:])

            def run_stage(b, s):
                if s == 0:
                    stage_embed(b)
                    g_phase(b, 0)
                elif s == 1 + 2 * L:
                    stage_out(b)
                elif s % 2 == 1:
                    stage_gather(b, (s - 1) // 2)
                else:
                    stage_update(b, s // 2 - 1)

            NSTAGE = 2 + 2 * L
            for step in range(NSTAGE + GPC - 1):
                for b in range(GPC):
                    s = step - b
                    if 0 <= s < NSTAGE:
                        run_stage(b, s)

    nc.compile()
    return nc


# --------------------------------------------------------------------------
# entry point
# --------------------------------------------------------------------------

def kernel(batch_nodes, batch_adj_tuples, subgraph_mask, num_subgraphs,
           type_table, depth_table, W_rel, b_conv, W_gate, b_gate,
           W_out, b_out):
    key, tot, idx_imgs, onehots, maskT = _preprocess(
        batch_nodes, batch_adj_tuples, subgraph_mask)
    sched, chunks, elide = key
    if key not in _compiled:
        _compiled[key] = _build(sched, chunks, tot, elide)
    nc = _compiled[key]

    bd = np.zeros((D, D), np.float32)
    bd[:TN, :TN] = np.asarray(type_table, np.float32)
    bd[TN:, TN:] = np.asarray(depth_table, np.float32)

    common = dict(
        blockdiag=bd,
        wrel=np.asarray(W_rel, np.float32).reshape(L * T * D, D),
        wgate=np.asarray(W_gate, np.float32).reshape(L * D, D),
        wout=np.asarray(W_out, np.float32).reshape(2 * D, D),
        bconvT=np.ascontiguousarray(np.asarray(b_conv, np.float32).T),
        bgateT=np.ascontiguousarray(np.asarray(b_gate, np.float32).T),
        boutT=np.ascontiguousarray(np.asarray(b_out, np.float32).T),
        b1rep=np.tile(np.asarray(b_out, np.float32)[1][None, :], (128, 1)),
        ident=np.eye(128, dtype=np.float32),
    )
    in_maps = [
        dict(common, idximg=idx_imgs[c], onehot=onehots[c], maskT=maskT[c])
        for c in range(NCORES)
    ]

    res = run_bass_kernel_spmd(nc, in_maps, list(range(NCORES)))
    out = np.concatenate([res.results[c]["gemb"] for c in range(NCORES)], 0)
    return out.astype(np.float32)
